# revision 13
# baseline (speedup 1.0000x reference)
"""Multi-head attention (B=4, N=2048, DIM=768, H=8, DH=96) on 8 TRN2 NeuronCores.

Sharding: data-parallel over (batch, query-half) — core c handles batch c//2,
query rows [(c%2)*1024, (c%2+1)*1024). Each core computes K/V for its full
batch (duplicated across the 2 cores sharing a batch), so there are NO
collectives: each core produces its own disjoint output shard.

Per-core compute (all matmuls bf16, fp32 PSUM accumulation):
  - Q^T/K^T projection in transposed space with head-dim padded 96->128 so
    each head's Q^T/K^T lands in its own 128-partition tiles.
  - V projection in natural space; a constant 1.0 column is appended per head
    (V|1) so the attn@V matmul also produces the softmax row-sums.
  - dots P^T[nk,nq]: lhsT=K^T[128dh, 128nk], rhs=Q^T[128dh, 512nq]; softmax
    scale folded into w_q host-side. exp() on ScalarE PSUM->SBUF(bf16), no
    max subtraction (logits max ~9, fp32-safe).
  - O'^T[97, nq] accumulated over 16 key tiles; row 96 = row-sum s.
  - normalize: evacuate O' to SBUF (frees PSUM fast), broadcast s with a K=1
    outer-product matmul, 1/s via reciprocal_approx_fast on the broadcast,
    multiply. All off the PE critical path.
  - proj in two 4-head stages: heads 0-3 projected mid-run (bias folded in),
    heads 4-7 + combine at the tail.

Emission is software-pipelined: Q/K/V projection chunks and stage-0 proj are
interleaved into the attention t-loops so ScalarE's exp stream never starves
and the PE always has ready work.

Output per core: y^T [768, 1024] fp32; host transposes/reassembles.
"""

import numpy as np
import ml_dtypes

B, N, DIM = 4, 2048, 768
H, DH = 8, 96
DHP = 128          # padded head dim for Q/K tiles
NQ = N // 2        # query rows per core
SCALE = DH ** -0.5
NCORES = 8
CT = DIM // 128    # 6 contraction chunks
NT = N // 128      # 16 key tiles
NQC = NQ // 512    # 2 query chunks of 512
NKC = N // 512     # 4 key chunks of 512

_CACHE = {}


def _build():
    import concourse.mybir as mybir
    import concourse.tile as tile
    from concourse import bacc

    f32 = mybir.dt.float32
    bf16 = mybir.dt.bfloat16
    Exp = mybir.ActivationFunctionType.Exp
    mult = mybir.AluOpType.mult
    add = mybir.AluOpType.add

    nc = bacc.Bacc("TRN2", debug=False, num_devices=NCORES)

    xt_d = nc.dram_tensor("xt", [DIM, N], bf16, kind="ExternalInput")
    wq_d = nc.dram_tensor("wq", [DIM, H * DHP], bf16, kind="ExternalInput")
    wk_d = nc.dram_tensor("wk", [DIM, H * DHP], bf16, kind="ExternalInput")
    wv_d = nc.dram_tensor("wv", [DIM, DIM], bf16, kind="ExternalInput")
    wp_d = nc.dram_tensor("wp", [H, DH, DIM], bf16, kind="ExternalInput")
    bias_d = nc.dram_tensor("bias", [DIM, 1], f32, kind="ExternalInput")
    out_d = nc.dram_tensor("out", [DIM, NQ], f32, kind="ExternalOutput")

    with tile.TileContext(nc) as tc:
        with (
            tc.tile_pool(name="const", bufs=1) as cpool,
            tc.tile_pool(name="ptp", bufs=4) as pt_pool,
            tc.tile_pool(name="onp", bufs=12) as on_pool,
            tc.tile_pool(name="smallp", bufs=2) as small_pool,
            tc.tile_pool(name="ysb", bufs=2) as y_pool,
            tc.tile_pool(name="ps_qkv", bufs=2, space="PSUM") as psum_qkv,
            tc.tile_pool(name="ps_d", bufs=3, space="PSUM") as psum_d,
            tc.tile_pool(name="ps_o", bufs=2, space="PSUM") as psum_o,
            tc.tile_pool(name="ps_rb", bufs=1, space="PSUM") as psum_rb,
        ):
            # ---- persistent SBUF tensors, consolidated input DMAs ----
            # xt split in two column halves so the first projections start
            # after ~1/4 of the input traffic.
            xt_a = cpool.tile([128, CT, NQ], bf16, name="xt_a")
            xt_b = cpool.tile([128, CT, NQ], bf16, name="xt_b")
            wk_sb = cpool.tile([128, CT, H * DHP], bf16, name="wk_sb")
            wq_sb = cpool.tile([128, CT, H * DHP], bf16, name="wq_sb")
            wv_sb = cpool.tile([128, CT, DIM], bf16, name="wv_sb")
            wp_sb = cpool.tile([DH, H, DIM], bf16, name="wp_sb")
            bias_sb = cpool.tile([128, CT, 1], f32, name="bias_sb")
            qt_sb = [
                [cpool.tile([DHP, 512], bf16, name=f"qt{h}_{qc}") for qc in range(NQC)]
                for h in range(H)
            ]
            kt_sb = [
                [cpool.tile([DHP, 512], bf16, name=f"kt{h}_{nc_}") for nc_ in range(NKC)]
                for h in range(H)
            ]
            v_sb = [cpool.tile([128, H, DH + 1], bf16, name=f"v{t}") for t in range(NT)]
            ones_sb = cpool.tile([128, DH], bf16, name="ones")
            y1_sb = [
                [cpool.tile([128, 512], bf16, name=f"y1_{ct}_{qc}") for qc in range(NQC)]
                for ct in range(CT)
            ]

            xt_r = xt_d.ap().rearrange("(c p) n -> p c n", p=128)
            nc.sync.dma_start(xt_a[:], xt_r[:, :, 0:NQ])
            nc.sync.dma_start(wk_sb[:], wk_d.ap().rearrange("(c p) f -> p c f", p=128))
            nc.sync.dma_start(wq_sb[:], wq_d.ap().rearrange("(c p) f -> p c f", p=128))
            nc.sync.dma_start(xt_b[:], xt_r[:, :, NQ:N])
            nc.sync.dma_start(wv_sb[:], wv_d.ap().rearrange("(c p) f -> p c f", p=128))
            nc.sync.dma_start(wp_sb[:], wp_d.ap().rearrange("h p n -> p h n"))
            nc.sync.dma_start(
                bias_sb[:], bias_d.ap().rearrange("(c p) o -> p c o", p=128)
            )

            nc.vector.memset(ones_sb[:], 1.0)
            for t in range(NT):
                nc.vector.memset(v_sb[t][:, :, DH:DH + 1], 1.0)

            def xt_cols(lo):
                # [128, CT-plane, 512] slice of the right xt half, lo in elems
                src = xt_a if lo < NQ else xt_b
                off = lo if lo < NQ else lo - NQ
                return src, off

            # ---- chunk emitters ----
            def k_chunk(h, nc_):
                src, off = xt_cols(nc_ * 512)
                ps = psum_qkv.tile([128, 512], f32, name="kps", tag="qkvps")
                for ct in range(CT):
                    nc.tensor.matmul(
                        ps,
                        lhsT=wk_sb[:, ct, h * DHP:(h + 1) * DHP],
                        rhs=src[:, ct, off:off + 512],
                        start=(ct == 0),
                        stop=(ct == CT - 1),
                    )
                nc.vector.tensor_copy(out=kt_sb[h][nc_][:], in_=ps[:])

            def q_chunk(h, qc):
                src, off = xt_cols(qc * 512)
                ps = psum_qkv.tile([128, 512], f32, name="qps", tag="qkvps")
                for ct in range(CT):
                    nc.tensor.matmul(
                        ps,
                        lhsT=wq_sb[:, ct, h * DHP:(h + 1) * DHP],
                        rhs=src[:, ct, off:off + 512],
                        start=(ct == 0),
                        stop=(ct == CT - 1),
                    )
                nc.vector.tensor_copy(out=qt_sb[h][qc][:], in_=ps[:])

            def v_chunk(t, fc):
                src, off = xt_cols(t * 128)
                ps = psum_qkv.tile([128, 512], f32, name="vps", tag="qkvps")
                vps = ps[:, :4 * DH]
                for ct in range(CT):
                    nc.tensor.matmul(
                        vps,
                        lhsT=src[:, ct, off:off + 128],
                        rhs=wv_sb[:, ct, fc * 4 * DH:(fc + 1) * 4 * DH],
                        start=(ct == 0),
                        stop=(ct == CT - 1),
                    )
                for j in range(4):
                    nc.vector.tensor_copy(
                        out=v_sb[t][:, fc * 4 + j, 0:DH],
                        in_=ps[:, j * DH:(j + 1) * DH],
                    )

            on_sb = {}

            def proj_stage(stage, ct, qc):
                """Project heads [4*stage, 4*stage+4) for output chunk (ct, qc)."""
                yp = psum_d.tile([128, 512], f32, name="yps", tag="dps")
                for i, h in enumerate(range(4 * stage, 4 * stage + 4)):
                    nc.tensor.matmul(
                        yp,
                        lhsT=wp_sb[:, h, ct * 128:(ct + 1) * 128],
                        rhs=on_sb[(h, qc)][:],
                        start=(i == 0),
                        stop=(i == 3),
                    )
                if stage == 0:
                    nc.vector.tensor_scalar_add(
                        y1_sb[ct][qc][:], yp[:], bias_sb[:, ct, :]
                    )
                else:
                    y_sb = y_pool.tile([128, 512], f32, name="y", tag="y")
                    nc.vector.tensor_tensor(y_sb[:], yp[:], y1_sb[ct][qc][:], add)
                    nc.sync.dma_start(
                        out_d.ap()[ct * 128:(ct + 1) * 128, qc * 512:(qc + 1) * 512],
                        y_sb[:],
                    )

            # ---- attention for one head, with fillers interleaved per slot ----
            def attn_head(h, fillers):
                o_ps = [
                    psum_o.tile([DH + 1, 512], f32, name=f"ops{qc}", tag="ops")
                    for qc in range(NQC)
                ]
                for t in range(NT):
                    for qc in range(NQC):
                        d_ps = psum_d.tile([128, 512], f32, name="dps", tag="dps")
                        nc.tensor.matmul(
                            d_ps[:],
                            lhsT=kt_sb[h][t // 4][:, (t % 4) * 128:(t % 4 + 1) * 128],
                            rhs=qt_sb[h][qc][:],
                            start=True,
                            stop=True,
                        )
                        pt = pt_pool.tile([128, 512], bf16, name="pt", tag="pt")
                        nc.scalar.activation(pt[:], d_ps[:], Exp)
                        nc.tensor.matmul(
                            o_ps[qc],
                            lhsT=v_sb[t][:, h, :],
                            rhs=pt[:],
                            start=(t == 0),
                            stop=(t == NT - 1),
                        )
                    for fn in fillers.get(t, ()):
                        fn()
                # normalize (off the PE critical path)
                for qc in range(NQC):
                    o_st = small_pool.tile(
                        [DH + 1, 512], f32, name="ostage", tag="ostage", bufs=3
                    )
                    nc.vector.tensor_copy(out=o_st[:], in_=o_ps[qc][:])
                    s16 = small_pool.tile([DH + 1, 512], bf16, name="s16", tag="s16")
                    nc.vector.tensor_copy(out=s16[DH:DH + 1, :], in_=o_st[DH:DH + 1, :])
                    s0 = small_pool.tile([1, 512], bf16, name="s0", tag="s0")
                    nc.sync.dma_start(s0[:], s16[DH:DH + 1, :])
                    rbs = psum_rb.tile([DH, 512], f32, name="rbs", tag="rb")
                    nc.tensor.matmul(
                        rbs[:], lhsT=ones_sb[0:1, :], rhs=s0[:], start=True, stop=True
                    )
                    rinv = small_pool.tile([DH, 512], f32, name="rinv", tag="rinv")
                    nc.vector.reciprocal_approx_fast(out=rinv[:], in_=rbs[:])
                    on = on_pool.tile([DH, 512], bf16, name="on", tag="on")
                    on_sb[(h, qc)] = on
                    nc.vector.tensor_tensor(on[:], o_st[0:DH, :], rinv[:], mult)

            # ---- software-pipelined emission ----
            k_chunk(0, 0)
            q_chunk(0, 0)
            q_chunk(0, 1)
            v_chunk(0, 0)
            v_chunk(1, 0)

            def mk_fillers(h):
                f = {}

                def addf(slot, fn):
                    f.setdefault(slot, []).append(fn)

                # remaining K chunks of this head, just-in-time
                for i, nc_ in enumerate((1, 2, 3)):
                    if h == 0:
                        addf(4 * nc_ - 3, (lambda hh=h, n=nc_: k_chunk(hh, n)))
                    else:
                        addf(2 * i, (lambda hh=h, n=nc_: k_chunk(hh, n)))
                # next head's first chunks late in this head
                if h + 1 < H:
                    addf(10, lambda hh=h + 1: k_chunk(hh, 0))
                    addf(12, lambda hh=h + 1: q_chunk(hh, 0))
                    addf(14, lambda hh=h + 1: q_chunk(hh, 1))
                # V tiles: head 0 consumes fc=0 just-in-time; fc=1 spread over
                # heads 1-3.
                if h == 0:
                    for t in range(2, NT):
                        addf(t - 2, lambda tt=t: v_chunk(tt, 0))
                elif h in (1, 2, 3):
                    start = [0, 6, 11][h - 1]
                    end = [6, 11, 16][h - 1]
                    slots = (1, 3, 5, 7, 9, 11)
                    for i, t in enumerate(range(start, end)):
                        addf(slots[i], lambda tt=t: v_chunk(tt, 1))
                # stage-0 projection (heads 0-3) spread over heads 4-5
                if h in (4, 5):
                    slots = (1, 3, 5, 7, 9, 11)
                    for i in range(6):
                        ct = (h - 4) * 3 + i // 2
                        qc = i % 2
                        addf(slots[i], lambda c=ct, q=qc: proj_stage(0, c, q))
                return f

            for h in range(H):
                attn_head(h, mk_fillers(h))

            # ---- tail: stage-1 projection (heads 4-7) + combine + out ----
            for ct in range(CT):
                for qc in range(NQC):
                    proj_stage(1, ct, qc)

    nc.compile()
    return nc


def _get_nc():
    if "nc" not in _CACHE:
        _CACHE["nc"] = _build()
    return _CACHE["nc"]


def _prep_shards(x, w_qkv, w_proj, b_proj):
    bf16 = ml_dtypes.bfloat16
    x = np.asarray(x, dtype=np.float32)
    w_qkv = np.asarray(w_qkv, dtype=np.float32)
    w_proj = np.asarray(w_proj, dtype=np.float32)
    b_proj = np.asarray(b_proj, dtype=np.float32)

    # w_qkv: [3*INNER, DIM] rows: q rows [h*96+d], k rows 768+..., v rows 1536+...
    wqT = w_qkv[0:DIM].T.reshape(DIM, H, DH)        # [c, h, d]
    wkT = w_qkv[DIM:2 * DIM].T.reshape(DIM, H, DH)
    wvT = w_qkv[2 * DIM:3 * DIM].T                  # [c, f] natural head-major
    wq_pad = np.zeros((DIM, H, DHP), np.float32)
    wk_pad = np.zeros((DIM, H, DHP), np.float32)
    wq_pad[:, :, :DH] = wqT * SCALE
    wk_pad[:, :, :DH] = wkT
    wq_b = np.ascontiguousarray(wq_pad.reshape(DIM, H * DHP)).astype(bf16)
    wk_b = np.ascontiguousarray(wk_pad.reshape(DIM, H * DHP)).astype(bf16)
    wv_b = np.ascontiguousarray(wvT).astype(bf16)
    wp_b = np.ascontiguousarray(w_proj.T.reshape(H, DH, DIM)).astype(bf16)
    bias = np.ascontiguousarray(b_proj.reshape(DIM, 1))

    in_maps = []
    for c in range(NCORES):
        b, half = divmod(c, 2)
        xt = x[b].T  # [768, 2048]
        if half == 1:
            xt = np.concatenate([xt[:, NQ:], xt[:, :NQ]], axis=1)
        in_maps.append({
            "xt": np.ascontiguousarray(xt).astype(bf16),
            "wq": wq_b,
            "wk": wk_b,
            "wv": wv_b,
            "wp": wp_b,
            "bias": bias,
        })
    return in_maps


def kernel(x, w_qkv, w_proj, b_proj):
    from concourse.bass_utils import run_bass_kernel_spmd

    nc = _get_nc()
    in_maps = _prep_shards(x, w_qkv, w_proj, b_proj)
    res = run_bass_kernel_spmd(nc, in_maps, core_ids=list(range(NCORES)))
    out = np.empty((B, N, DIM), np.float32)
    for c in range(NCORES):
        b, half = divmod(c, 2)
        yT = np.asarray(res.results[c]["out"], dtype=np.float32)  # [768, 1024]
        out[b, half * NQ:(half + 1) * NQ, :] = yT.T
    return out
